# revision 13
# baseline (speedup 1.0000x reference)
"""Single-head attention layer (embed 1024) on 8 trn2 NeuronCores.

Reference (per batch b of 4, torch-Linear convention):
    Q = x@Wq.T + bq ; K = x@Wk.T + bk ; V = x@Wv.T + bv
    S = Q@K.T / 32 ; P = softmax(S, -1) ; out = (P@V)@Wo.T + bo

Sharding: data-parallel over (batch, seq-half): core c handles batch c//2,
query rows [(c%2)*1024, +1024). K/V work for a batch is duplicated on the two
cores sharing it (cheaper than collectives).

Per-core dataflow (all matmuls fp32r = full-rate PE with ~1e-4 rel err):
    Q^T[d,q]   = accum_e  WqT[e,d-tile].T @ xTq[e,q]          (+bq)
    per k-block of 256 (8 blocks):
      K^T[d,k] = accum_e  WkT[e,d-tile].T @ xT[e,k-blk]
      V[k,d]   = accum_e  xT[e,k-tile].T  @ WvT[e,d]
      S^T[k,q] = accum_d  K^T[d,k-tile].T @ Q^T[d,q]
      E        = exp(S^T/32)                                   (ACT, fused scale)
      sums[q] += ones.T @ E                                    (PSUM accum, all blocks)
      O^T[d,q]+= V[k,d-tile].T @ E[k,q]                        (SBUF accum via DVE)
    Y[q,f]     = accum_d  O^T[d,q-tile].T @ WoT[d,f]
    y          = Y * (1/sums[q]) + bout                        (per-partition scale)

Bias handling (exact): bk shifts every score in a row equally -> softmax
invariant -> dropped. bv flows through P@V as a constant row (rows of P sum
to 1) -> folded host-side into bout = Wo@bv + bo. bq is applied to Q^T
(per-partition add). Softmax skips max-subtraction: scores ~ N(0,1), exp
overflow needs |s|>88.

Host prep: x and the four weights are transposed host-side so every DMA is a
natural row-major load (the TensorE contracts over the partition dim).
"""

import numpy as np

import concourse.bass as bass
import concourse.mybir as mybir
from concourse.tile import TileContext
from concourse.vector_clock import ScopedClock
from concourse import bass_utils

F32 = mybir.dt.float32
F32R = mybir.dt.float32r
AF = mybir.ActivationFunctionType
ALU = mybir.AluOpType

D = 1024          # embed dim
S = 2048          # seq len
QC = 1024         # query rows per core
KB = 256          # k-block size
NBLK = S // KB    # 8
P = 128           # partitions
SCALE = 1.0 / 32.0


# ---------------------------------------------------------------------------
# This container's walrus build rejects >1 sync-wait per instruction
# ("Too many sync wait commands"). Split multi-waits onto preceding
# same-engine NOPs: waits gate dispatch and engine queues are FIFO, so this
# is semantically identical.
_wait_split_installed = False


def _install_wait_split():
    global _wait_split_installed
    if _wait_split_installed:
        return
    _wait_split_installed = True

    orig_commit = TileContext._commit_instruction

    def commit(self, inst, lazy_reg_writes=True):
        si = getattr(inst, "sync_info", None)
        if si is not None:
            waits = list(si.on_wait or [])
            updates = list(si.on_update or [])
            if len(updates) > 1:
                raise RuntimeError(
                    f"{inst.name}: {len(updates)} sem updates unsupported"
                )
            if len(waits) > 1:
                for w in waits[:-1]:
                    nop = mybir.InstNoOp(
                        name=f"waitsplit_{self.nc.next_id()}",
                        engine=inst.engine,
                        bass_nofuse=True,
                        sync_info=mybir.SyncInfo(on_wait=[w], on_update=[]),
                    )
                    orig_commit(self, nop, lazy_reg_writes)
                inst.sync_info = mybir.SyncInfo(on_wait=[waits[-1]], on_update=updates)
        return orig_commit(self, inst, lazy_reg_writes)

    def drain_and_barrier(self, tick_clock, wait_clock):
        stage = self.nc.sync.nop(nofuse=True)
        wait_clock.add_sem_waits(
            stage.ins, ScopedClock({None: tick_clock.global_clock})
        )
        si = stage.ins.sync_info
        if si is not None and len(si.on_wait or []) > 1:
            waits = list(si.on_wait)
            stage.ins.sync_info = mybir.SyncInfo(
                on_wait=[waits[0]], on_update=list(si.on_update or [])
            )
            for w in waits[1:]:
                extra = self.nc.sync.nop(nofuse=True)
                extra.ins.sync_info = mybir.SyncInfo(on_wait=[w], on_update=[])
        self.nc.sync.drain()
        self.nc.all_engine_barrier()
        assert self.sems is not None
        popped = self.nc._tile_sem_poison_stack.pop()
        assert popped is self._sem_poison
        self.nc.clear_and_free_semaphores(list(self.sems.allocated().values()))
        self.nc.all_engine_barrier()

    TileContext._commit_instruction = commit
    TileContext._drain_and_barrier = drain_and_barrier


def _build():
    nc = bass.Bass()
    xT = nc.dram_tensor("xT", [D, S], F32R, kind="ExternalInput")
    xTq = nc.dram_tensor("xTq", [D, QC], F32R, kind="ExternalInput")
    wqT = nc.dram_tensor("wqT", [D, D], F32R, kind="ExternalInput")
    wkT = nc.dram_tensor("wkT", [D, D], F32R, kind="ExternalInput")
    wvT = nc.dram_tensor("wvT", [D, D], F32R, kind="ExternalInput")
    woT = nc.dram_tensor("woT", [D, D], F32R, kind="ExternalInput")
    ones_d = nc.dram_tensor("ones", [P, 1], F32R, kind="ExternalInput")
    bq_d = nc.dram_tensor("bq", [D], F32, kind="ExternalInput")
    bout_d = nc.dram_tensor("bout", [D], F32, kind="ExternalInput")
    y_d = nc.dram_tensor("y", [QC, D], F32, kind="ExternalOutput")
    sums_scratch = nc.dram_tensor("sums_scratch", [QC], F32, kind="Internal")

    ET = D // P    # 8 e-tiles
    DT = D // P    # 8 d-tiles
    QT = QC // P   # 8 q-tiles
    NQ = QC // 512  # 2 q chunks of 512
    KTB = KB // P  # 2 k-tiles per block

    def row_tiles(dram, width):
        # [D, width] dram -> [128, D//128, width] sbuf AP pattern
        return dram.ap().rearrange("(t p) w -> p t w", p=P)

    with TileContext(nc) as tc:
        with (
            tc.tile_pool(name="consts", bufs=1) as consts,
            tc.tile_pool(name="persist", bufs=1) as persist,
            tc.tile_pool(name="ps_main", bufs=4, space="PSUM") as ps_main,
            tc.tile_pool(name="ps_sums", bufs=1, space="PSUM") as ps_sums,
        ):
            ones_t = consts.tile([P, 1], F32R)
            nc.sync.dma_start(out=ones_t, in_=ones_d.ap())
            bq_t = consts.tile([P, DT], F32)
            nc.sync.dma_start(out=bq_t, in_=bq_d.ap().rearrange("(t p) -> p t", p=P))
            # bout broadcast to all partitions (DVE ops need real APs, so
            # materialize the broadcast via DMA)
            bout_t = consts.tile([P, D], F32)
            _bd = bout_d.ap()
            nc.gpsimd.dma_start(
                out=bout_t,
                in_=bass.AP(tensor=_bd.tensor, offset=_bd.offset,
                            ap=[[0, P], *_bd.ap]),
            )

            qT_sb = persist.tile([P, DT, QC], F32R)    # Q^T: [d-part, d-tile, q]
            oT_sb = persist.tile([P, DT, QC], F32R)    # O^T accum (unnormalized)

            sums_ps = [ps_sums.tile([1, 512], F32, name=f"sums_ps{i}") for i in range(NQ)]

            # ---- Phase A: Q^T = WqT.T @ xTq (+bq) --------------------------
            with tc.tile_pool(name="ph_a", bufs=1) as ph_a:
                wq_sb = ph_a.tile([P, ET, D], F32R)
                nc.sync.dma_start(out=wq_sb, in_=row_tiles(wqT, D))
                xq_sb = ph_a.tile([P, ET, QC], F32R)
                nc.sync.dma_start(out=xq_sb, in_=row_tiles(xTq, QC))
                for td in range(DT):
                    for qc in range(NQ):
                        q_ps = ps_main.tile([P, 512], F32, name="q_ps", tag="mm")
                        for te in range(ET):
                            nc.tensor.matmul(
                                q_ps[:],
                                wq_sb[:, te, td * P:(td + 1) * P],
                                xq_sb[:, te, qc * 512:(qc + 1) * 512],
                                start=(te == 0),
                                stop=(te == ET - 1),
                            )
                        nc.vector.tensor_scalar_add(
                            qT_sb[:, td, qc * 512:(qc + 1) * 512],
                            q_ps[:],
                            bq_t[:, td:td + 1],
                        )

            # ---- Phase B: stream k-blocks ----------------------------------
            with tc.tile_pool(name="ph_b_w", bufs=1) as ph_b_w:
                wk_sb = ph_b_w.tile([P, ET, D], F32R)
                nc.sync.dma_start(out=wk_sb, in_=row_tiles(wkT, D))
                wv_sb = ph_b_w.tile([P, ET, D], F32R)
                nc.sync.dma_start(out=wv_sb, in_=row_tiles(wvT, D))

                with tc.tile_pool(name="ph_b_blk", bufs=2) as blk:
                    xT_view = row_tiles(xT, S)
                    for kb in range(NBLK):
                        k0 = kb * KB
                        xb_sb = blk.tile([P, ET, KB], F32R, name="xb_sb")
                        nc.sync.dma_start(
                            out=xb_sb, in_=xT_view[:, :, k0:k0 + KB]
                        )

                        # K^T block: [d-part, d-tile, k-in-block]
                        kT_sb = blk.tile([P, DT, KB], F32R, name="kT_sb")
                        for td in range(DT):
                            k_ps = ps_main.tile([P, KB], F32, name="k_ps", tag="mm")
                            for te in range(ET):
                                nc.tensor.matmul(
                                    k_ps[:],
                                    wk_sb[:, te, td * P:(td + 1) * P],
                                    xb_sb[:, te, :],
                                    start=(te == 0),
                                    stop=(te == ET - 1),
                                )
                            nc.vector.tensor_copy(kT_sb[:, td, :], k_ps[:])

                        # V block: [k-part, k-tile, d]
                        v_sb = blk.tile([P, KTB, D], F32R, name="v_sb")
                        for tk in range(KTB):
                            for dc in range(NQ):
                                v_ps = ps_main.tile([P, 512], F32, name="v_ps", tag="mm")
                                for te in range(ET):
                                    nc.tensor.matmul(
                                        v_ps[:],
                                        xb_sb[:, te, tk * P:(tk + 1) * P],
                                        wv_sb[:, te, dc * 512:(dc + 1) * 512],
                                        start=(te == 0),
                                        stop=(te == ET - 1),
                                    )
                                nc.vector.tensor_copy(
                                    v_sb[:, tk, dc * 512:(dc + 1) * 512], v_ps[:]
                                )

                        # S^T block + exp -> E
                        e_sb = blk.tile([P, KTB, QC], F32R, name="e_sb")
                        for tk in range(KTB):
                            for qc in range(NQ):
                                s_ps = ps_main.tile([P, 512], F32, name="s_ps", tag="mm")
                                for td in range(DT):
                                    nc.tensor.matmul(
                                        s_ps[:],
                                        kT_sb[:, td, tk * P:(tk + 1) * P],
                                        qT_sb[:, td, qc * 512:(qc + 1) * 512],
                                        start=(td == 0),
                                        stop=(td == DT - 1),
                                    )
                                nc.scalar.activation(
                                    e_sb[:, tk, qc * 512:(qc + 1) * 512],
                                    s_ps[:],
                                    AF.Exp,
                                    scale=SCALE,
                                )
                                # running softmax denominator (PSUM accum
                                # across all blocks)
                                nc.tensor.matmul(
                                    sums_ps[qc][:],
                                    ones_t[:],
                                    e_sb[:, tk, qc * 512:(qc + 1) * 512],
                                    start=(kb == 0 and tk == 0),
                                    stop=(kb == NBLK - 1 and tk == KTB - 1),
                                    skip_group_check=True,
                                )

                        # O^T += V.T @ E  (accumulate in SBUF via DVE)
                        for td in range(DT):
                            for qc in range(NQ):
                                o_ps = ps_main.tile([P, 512], F32, name="o_ps", tag="mm")
                                for tk in range(KTB):
                                    nc.tensor.matmul(
                                        o_ps[:],
                                        v_sb[:, tk, td * P:(td + 1) * P],
                                        e_sb[:, tk, qc * 512:(qc + 1) * 512],
                                        start=(tk == 0),
                                        stop=(tk == KTB - 1),
                                    )
                                dst = oT_sb[:, td, qc * 512:(qc + 1) * 512]
                                if kb == 0:
                                    nc.vector.tensor_copy(dst, o_ps[:])
                                else:
                                    nc.vector.tensor_add(dst, dst, o_ps[:])

            # ---- Phase C: Y = O^T.T @ WoT, normalize, bias -----------------
            with tc.tile_pool(name="ph_c", bufs=1) as ph_c:
                wo_sb = ph_c.tile([P, DT, D], F32R)
                nc.sync.dma_start(out=wo_sb, in_=row_tiles(woT, D))

                sums_sb = ph_c.tile([1, QC], F32)
                for qc in range(NQ):
                    nc.vector.tensor_copy(
                        sums_sb[0:1, qc * 512:(qc + 1) * 512], sums_ps[qc][:]
                    )
                # transpose [1, 1024] -> [128, 8] via DRAM bounce
                # (column t = q-tile t)
                nc.sync.dma_start(
                    out=sums_scratch.ap().rearrange("(a q) -> a q", a=1),
                    in_=sums_sb[0:1, :],
                )
                sumsT_sb = ph_c.tile([P, QT], F32)
                nc.sync.dma_start(
                    out=sumsT_sb,
                    in_=sums_scratch.ap().rearrange("(t p) -> p t", p=P),
                )
                inv_sb = ph_c.tile([P, QT], F32)
                nc.vector.reciprocal(inv_sb[:], sumsT_sb[:])

                with tc.tile_pool(name="ph_c_out", bufs=3) as ph_c_out:
                    for tq in range(QT):
                        for fc in range(NQ):
                            y_ps = ps_main.tile([P, 512], F32, name="y_ps", tag="mm")
                            for td in range(DT):
                                nc.tensor.matmul(
                                    y_ps[:],
                                    oT_sb[:, td, tq * P:(tq + 1) * P],
                                    wo_sb[:, td, fc * 512:(fc + 1) * 512],
                                    start=(td == 0),
                                    stop=(td == DT - 1),
                                )
                            y_sb = ph_c_out.tile([P, 512], F32, name="y_sb")
                            nc.vector.tensor_scalar_mul(
                                y_sb[:], y_ps[:], inv_sb[:, tq:tq + 1]
                            )
                            nc.vector.tensor_add(
                                y_sb[:],
                                y_sb[:],
                                bout_t[:, fc * 512:(fc + 1) * 512],
                            )
                            nc.sync.dma_start(
                                out=y_d[tq * P:(tq + 1) * P,
                                        fc * 512:(fc + 1) * 512],
                                in_=y_sb[:],
                            )
    return nc


_nc_cache = None


def _build_in_maps(x, Wq, bq, Wk, bk, Wv, bv, Wo, bo):
    x = np.asarray(x, np.float32)
    Wq = np.asarray(Wq, np.float32)
    Wk = np.asarray(Wk, np.float32)
    Wv = np.asarray(Wv, np.float32)
    Wo = np.asarray(Wo, np.float32)
    bq = np.asarray(bq, np.float32)
    bv = np.asarray(bv, np.float32)
    bo = np.asarray(bo, np.float32)

    xT = np.ascontiguousarray(x.transpose(0, 2, 1))          # [B, D, S]
    wqT = np.ascontiguousarray(Wq.T)
    wkT = np.ascontiguousarray(Wk.T)
    wvT = np.ascontiguousarray(Wv.T)
    woT = np.ascontiguousarray(Wo.T)
    bout = (Wo @ bv + bo).astype(np.float32)

    in_maps = []
    for c in range(8):
        b, half = c // 2, c % 2
        q0 = half * QC
        in_maps.append({
            "xT": xT[b],
            "xTq": np.ascontiguousarray(xT[b][:, q0:q0 + QC]),
            "wqT": wqT, "wkT": wkT, "wvT": wvT, "woT": woT,
            "bq": bq, "bout": bout,
            "ones": np.ones((P, 1), np.float32),
        })
    return in_maps


def get_nc():
    global _nc_cache
    _install_wait_split()
    if _nc_cache is None:
        _nc_cache = _build()
    return _nc_cache


def kernel(x, Wq, bq, Wk, bk, Wv, bv, Wo, bo, **run_kwargs):
    B = np.asarray(x).shape[0]
    in_maps = _build_in_maps(x, Wq, bq, Wk, bk, Wv, bv, Wo, bo)
    res = bass_utils.run_bass_kernel_spmd(
        get_nc(), in_maps, core_ids=list(range(8)), **run_kwargs
    )
    if run_kwargs:
        kernel.last_result = res
    y = np.empty((B, S, D), np.float32)
    for c in range(8):
        b, half = c // 2, c % 2
        y[b, half * QC:(half + 1) * QC, :] = res.results[c]["y"]
    return y


# revision 14
# speedup vs baseline: 1.0625x; 1.0625x over previous
"""Single-head attention layer (embed 1024) on 8 trn2 NeuronCores.

Reference (per batch b of 4, torch-Linear convention):
    Q = x@Wq.T + bq ; K = x@Wk.T + bk ; V = x@Wv.T + bv
    S = Q@K.T / 32 ; P = softmax(S, -1) ; out = (P@V)@Wo.T + bo

Sharding: data-parallel over (batch, seq-half): core c handles batch c//2,
query rows [(c%2)*1024, +1024). K/V work for a batch is duplicated on the two
cores sharing it (cheaper than collectives).

Per-core dataflow (all matmuls fp32r = full-rate PE with ~1e-4 rel err):
    Q^T[d,q]   = accum_e  WqT[e,d-tile].T @ xTq[e,q]          (+bq)
    per k-block of 256 (8 blocks):
      K^T[d,k] = accum_e  WkT[e,d-tile].T @ xT[e,k-blk]
      V[k,d]   = accum_e  xT[e,k-tile].T  @ WvT[e,d]
      S^T[k,q] = accum_d  K^T[d,k-tile].T @ Q^T[d,q]
      E        = exp(S^T/32)                                   (ACT, fused scale)
      sums[q] += ones.T @ E                                    (PSUM accum, all blocks)
      O^T[d,q]+= V[k,d-tile].T @ E[k,q]                        (SBUF accum via DVE)
    Y[q,f]     = accum_d  O^T[d,q-tile].T @ WoT[d,f]
    y          = Y * (1/sums[q]) + bout                        (per-partition scale)

Bias handling (exact): bk shifts every score in a row equally -> softmax
invariant -> dropped. bv flows through P@V as a constant row (rows of P sum
to 1) -> folded host-side into bout = Wo@bv + bo. bq is applied to Q^T
(per-partition add). Softmax skips max-subtraction: scores ~ N(0,1), exp
overflow needs |s|>88.

Host prep: x and the four weights are transposed host-side so every DMA is a
natural row-major load (the TensorE contracts over the partition dim).
"""

import numpy as np

import concourse.bass as bass
import concourse.mybir as mybir
from concourse.tile import TileContext
from concourse.vector_clock import ScopedClock
from concourse import bass_utils

F32 = mybir.dt.float32
F32R = mybir.dt.float32r
AF = mybir.ActivationFunctionType
ALU = mybir.AluOpType

D = 1024          # embed dim
S = 2048          # seq len
QC = 1024         # query rows per core
KB = 256          # k-block size
NBLK = S // KB    # 8
P = 128           # partitions
SCALE = 1.0 / 32.0


# ---------------------------------------------------------------------------
# This container's walrus build rejects >1 sync-wait per instruction
# ("Too many sync wait commands"). Split multi-waits onto preceding
# same-engine NOPs: waits gate dispatch and engine queues are FIFO, so this
# is semantically identical.
_wait_split_installed = False


def _install_wait_split():
    global _wait_split_installed
    if _wait_split_installed:
        return
    _wait_split_installed = True

    orig_commit = TileContext._commit_instruction

    def commit(self, inst, lazy_reg_writes=True):
        si = getattr(inst, "sync_info", None)
        if si is not None:
            waits = list(si.on_wait or [])
            updates = list(si.on_update or [])
            if len(updates) > 1:
                raise RuntimeError(
                    f"{inst.name}: {len(updates)} sem updates unsupported"
                )
            if len(waits) > 1:
                for w in waits[:-1]:
                    nop = mybir.InstNoOp(
                        name=f"waitsplit_{self.nc.next_id()}",
                        engine=inst.engine,
                        bass_nofuse=True,
                        sync_info=mybir.SyncInfo(on_wait=[w], on_update=[]),
                    )
                    orig_commit(self, nop, lazy_reg_writes)
                inst.sync_info = mybir.SyncInfo(on_wait=[waits[-1]], on_update=updates)
        return orig_commit(self, inst, lazy_reg_writes)

    def drain_and_barrier(self, tick_clock, wait_clock):
        stage = self.nc.sync.nop(nofuse=True)
        wait_clock.add_sem_waits(
            stage.ins, ScopedClock({None: tick_clock.global_clock})
        )
        si = stage.ins.sync_info
        if si is not None and len(si.on_wait or []) > 1:
            waits = list(si.on_wait)
            stage.ins.sync_info = mybir.SyncInfo(
                on_wait=[waits[0]], on_update=list(si.on_update or [])
            )
            for w in waits[1:]:
                extra = self.nc.sync.nop(nofuse=True)
                extra.ins.sync_info = mybir.SyncInfo(on_wait=[w], on_update=[])
        self.nc.sync.drain()
        self.nc.all_engine_barrier()
        assert self.sems is not None
        popped = self.nc._tile_sem_poison_stack.pop()
        assert popped is self._sem_poison
        self.nc.clear_and_free_semaphores(list(self.sems.allocated().values()))
        self.nc.all_engine_barrier()

    TileContext._commit_instruction = commit
    TileContext._drain_and_barrier = drain_and_barrier


def _build():
    nc = bass.Bass()
    xT = nc.dram_tensor("xT", [D, S], F32R, kind="ExternalInput")
    xTq = nc.dram_tensor("xTq", [D, QC], F32R, kind="ExternalInput")
    wqT = nc.dram_tensor("wqT", [D, D], F32R, kind="ExternalInput")
    wkT = nc.dram_tensor("wkT", [D, D], F32R, kind="ExternalInput")
    wvT = nc.dram_tensor("wvT", [D, D], F32R, kind="ExternalInput")
    woT = nc.dram_tensor("woT", [D, D], F32R, kind="ExternalInput")
    ones_d = nc.dram_tensor("ones", [P, 1], F32R, kind="ExternalInput")
    bq_d = nc.dram_tensor("bq", [D], F32, kind="ExternalInput")
    bout_d = nc.dram_tensor("bout", [D], F32, kind="ExternalInput")
    y_d = nc.dram_tensor("y", [QC, D], F32, kind="ExternalOutput")
    sums_scratch = nc.dram_tensor("sums_scratch", [QC], F32, kind="Internal")

    ET = D // P    # 8 e-tiles
    DT = D // P    # 8 d-tiles
    QT = QC // P   # 8 q-tiles
    NQ = QC // 512  # 2 q chunks of 512
    KTB = KB // P  # 2 k-tiles per block

    def row_tiles(dram, width):
        # [D, width] dram -> [128, D//128, width] sbuf AP pattern
        return dram.ap().rearrange("(t p) w -> p t w", p=P)

    with TileContext(nc) as tc:
        with (
            tc.tile_pool(name="consts", bufs=1) as consts,
            tc.tile_pool(name="persist", bufs=1) as persist,
            tc.tile_pool(name="ps_main", bufs=6, space="PSUM") as ps_main,
            tc.tile_pool(name="ps_sums", bufs=1, space="PSUM") as ps_sums,
        ):
            ones_t = consts.tile([P, 1], F32R)
            nc.sync.dma_start(out=ones_t, in_=ones_d.ap())
            bq_t = consts.tile([P, DT], F32)
            nc.sync.dma_start(out=bq_t, in_=bq_d.ap().rearrange("(t p) -> p t", p=P))
            # bout broadcast to all partitions (DVE ops need real APs, so
            # materialize the broadcast via DMA)
            bout_t = consts.tile([P, D], F32)
            _bd = bout_d.ap()
            nc.gpsimd.dma_start(
                out=bout_t,
                in_=bass.AP(tensor=_bd.tensor, offset=_bd.offset,
                            ap=[[0, P], *_bd.ap]),
            )

            qT_sb = persist.tile([P, DT, QC], F32R)    # Q^T: [d-part, d-tile, q]
            oT_sb = persist.tile([P, DT, QC], F32R)    # O^T accum (unnormalized)

            sums_ps = [ps_sums.tile([1, 512], F32, name=f"sums_ps{i}") for i in range(NQ)]

            # ---- Phase A: Q^T = WqT.T @ xTq (+bq) --------------------------
            with tc.tile_pool(name="ph_a", bufs=1) as ph_a:
                wq_sb = ph_a.tile([P, ET, D], F32R)
                xq_sb = ph_a.tile([P, ET, QC], F32R)
                for te in range(ET):
                    nc.sync.dma_start(
                        out=wq_sb[:, te, :], in_=row_tiles(wqT, D)[:, te, :]
                    )
                    nc.sync.dma_start(
                        out=xq_sb[:, te, :], in_=row_tiles(xTq, QC)[:, te, :]
                    )
                for td in range(DT):
                    for qc in range(NQ):
                        q_ps = ps_main.tile([P, 512], F32, name="q_ps", tag="mm")
                        for te in range(ET):
                            nc.tensor.matmul(
                                q_ps[:],
                                wq_sb[:, te, td * P:(td + 1) * P],
                                xq_sb[:, te, qc * 512:(qc + 1) * 512],
                                start=(te == 0),
                                stop=(te == ET - 1),
                            )
                        nc.vector.tensor_scalar_add(
                            qT_sb[:, td, qc * 512:(qc + 1) * 512],
                            q_ps[:],
                            bq_t[:, td:td + 1],
                        )

            # ---- Phase B: stream k-blocks ----------------------------------
            with tc.tile_pool(name="ph_b_w", bufs=1) as ph_b_w:
                wk_sb = ph_b_w.tile([P, ET, D], F32R)
                wv_sb = ph_b_w.tile([P, ET, D], F32R)
                for te in range(ET):
                    nc.sync.dma_start(
                        out=wk_sb[:, te, :], in_=row_tiles(wkT, D)[:, te, :]
                    )
                    nc.sync.dma_start(
                        out=wv_sb[:, te, :], in_=row_tiles(wvT, D)[:, te, :]
                    )

                with tc.tile_pool(name="ph_b_blk", bufs=2) as blk:
                    xT_view = row_tiles(xT, S)
                    for kb in range(NBLK):
                        k0 = kb * KB
                        xb_sb = blk.tile([P, ET, KB], F32R, name="xb_sb")
                        nc.sync.dma_start(
                            out=xb_sb, in_=xT_view[:, :, k0:k0 + KB]
                        )

                        # K^T block: [d-part, d-tile, k-in-block]
                        kT_sb = blk.tile([P, DT, KB], F32R, name="kT_sb")
                        for td in range(DT):
                            k_ps = ps_main.tile([P, KB], F32, name="k_ps", tag="mm")
                            for te in range(ET):
                                nc.tensor.matmul(
                                    k_ps[:],
                                    wk_sb[:, te, td * P:(td + 1) * P],
                                    xb_sb[:, te, :],
                                    start=(te == 0),
                                    stop=(te == ET - 1),
                                )
                            nc.scalar.activation(kT_sb[:, td, :], k_ps[:], AF.Copy)

                        # V block: [k-part, k-tile, d]
                        v_sb = blk.tile([P, KTB, D], F32R, name="v_sb")
                        for tk in range(KTB):
                            for dc in range(NQ):
                                v_ps = ps_main.tile([P, 512], F32, name="v_ps", tag="mm")
                                for te in range(ET):
                                    nc.tensor.matmul(
                                        v_ps[:],
                                        xb_sb[:, te, tk * P:(tk + 1) * P],
                                        wv_sb[:, te, dc * 512:(dc + 1) * 512],
                                        start=(te == 0),
                                        stop=(te == ET - 1),
                                    )
                                nc.scalar.activation(
                                    v_sb[:, tk, dc * 512:(dc + 1) * 512],
                                    v_ps[:], AF.Copy,
                                )

                        # S^T block + exp -> E
                        e_sb = blk.tile([P, KTB, QC], F32R, name="e_sb")
                        for tk in range(KTB):
                            for qc in range(NQ):
                                s_ps = ps_main.tile([P, 512], F32, name="s_ps", tag="mm")
                                for td in range(DT):
                                    nc.tensor.matmul(
                                        s_ps[:],
                                        kT_sb[:, td, tk * P:(tk + 1) * P],
                                        qT_sb[:, td, qc * 512:(qc + 1) * 512],
                                        start=(td == 0),
                                        stop=(td == DT - 1),
                                    )
                                nc.scalar.activation(
                                    e_sb[:, tk, qc * 512:(qc + 1) * 512],
                                    s_ps[:],
                                    AF.Exp,
                                    scale=SCALE,
                                )
                                # running softmax denominator (PSUM accum
                                # across all blocks)
                                nc.tensor.matmul(
                                    sums_ps[qc][:],
                                    ones_t[:],
                                    e_sb[:, tk, qc * 512:(qc + 1) * 512],
                                    start=(kb == 0 and tk == 0),
                                    stop=(kb == NBLK - 1 and tk == KTB - 1),
                                    skip_group_check=True,
                                )

                        # O^T += V.T @ E  (accumulate in SBUF via DVE)
                        for td in range(DT):
                            for qc in range(NQ):
                                o_ps = ps_main.tile([P, 512], F32, name="o_ps", tag="mm")
                                for tk in range(KTB):
                                    nc.tensor.matmul(
                                        o_ps[:],
                                        v_sb[:, tk, td * P:(td + 1) * P],
                                        e_sb[:, tk, qc * 512:(qc + 1) * 512],
                                        start=(tk == 0),
                                        stop=(tk == KTB - 1),
                                    )
                                dst = oT_sb[:, td, qc * 512:(qc + 1) * 512]
                                if kb == 0:
                                    nc.vector.tensor_copy(dst, o_ps[:])
                                else:
                                    nc.vector.tensor_add(dst, dst, o_ps[:])

            # ---- Phase C: Y = O^T.T @ WoT, normalize, bias -----------------
            with tc.tile_pool(name="ph_c", bufs=1) as ph_c:
                wo_sb = ph_c.tile([P, DT, D], F32R)
                for td in range(DT):
                    nc.sync.dma_start(
                        out=wo_sb[:, td, :], in_=row_tiles(woT, D)[:, td, :]
                    )

                sums_sb = ph_c.tile([1, QC], F32)
                for qc in range(NQ):
                    nc.vector.tensor_copy(
                        sums_sb[0:1, qc * 512:(qc + 1) * 512], sums_ps[qc][:]
                    )
                # transpose [1, 1024] -> [128, 8] via DRAM bounce
                # (column t = q-tile t)
                nc.sync.dma_start(
                    out=sums_scratch.ap().rearrange("(a q) -> a q", a=1),
                    in_=sums_sb[0:1, :],
                )
                sumsT_sb = ph_c.tile([P, QT], F32)
                nc.sync.dma_start(
                    out=sumsT_sb,
                    in_=sums_scratch.ap().rearrange("(t p) -> p t", p=P),
                )
                inv_sb = ph_c.tile([P, QT], F32)
                nc.vector.reciprocal(inv_sb[:], sumsT_sb[:])

                with tc.tile_pool(name="ph_c_out", bufs=3) as ph_c_out:
                    for tq in range(QT):
                        for fc in range(NQ):
                            y_ps = ps_main.tile([P, 512], F32, name="y_ps", tag="mm")
                            for td in range(DT):
                                nc.tensor.matmul(
                                    y_ps[:],
                                    oT_sb[:, td, tq * P:(tq + 1) * P],
                                    wo_sb[:, td, fc * 512:(fc + 1) * 512],
                                    start=(td == 0),
                                    stop=(td == DT - 1),
                                )
                            y_sb = ph_c_out.tile([P, 512], F32, name="y_sb")
                            nc.vector.tensor_scalar_mul(
                                y_sb[:], y_ps[:], inv_sb[:, tq:tq + 1]
                            )
                            nc.vector.tensor_add(
                                y_sb[:],
                                y_sb[:],
                                bout_t[:, fc * 512:(fc + 1) * 512],
                            )
                            nc.sync.dma_start(
                                out=y_d[tq * P:(tq + 1) * P,
                                        fc * 512:(fc + 1) * 512],
                                in_=y_sb[:],
                            )
    return nc


_nc_cache = None


def _build_in_maps(x, Wq, bq, Wk, bk, Wv, bv, Wo, bo):
    x = np.asarray(x, np.float32)
    Wq = np.asarray(Wq, np.float32)
    Wk = np.asarray(Wk, np.float32)
    Wv = np.asarray(Wv, np.float32)
    Wo = np.asarray(Wo, np.float32)
    bq = np.asarray(bq, np.float32)
    bv = np.asarray(bv, np.float32)
    bo = np.asarray(bo, np.float32)

    xT = np.ascontiguousarray(x.transpose(0, 2, 1))          # [B, D, S]
    wqT = np.ascontiguousarray(Wq.T)
    wkT = np.ascontiguousarray(Wk.T)
    wvT = np.ascontiguousarray(Wv.T)
    woT = np.ascontiguousarray(Wo.T)
    bout = (Wo @ bv + bo).astype(np.float32)

    in_maps = []
    for c in range(8):
        b, half = c // 2, c % 2
        q0 = half * QC
        in_maps.append({
            "xT": xT[b],
            "xTq": np.ascontiguousarray(xT[b][:, q0:q0 + QC]),
            "wqT": wqT, "wkT": wkT, "wvT": wvT, "woT": woT,
            "bq": bq, "bout": bout,
            "ones": np.ones((P, 1), np.float32),
        })
    return in_maps


def get_nc():
    global _nc_cache
    _install_wait_split()
    if _nc_cache is None:
        _nc_cache = _build()
    return _nc_cache


def kernel(x, Wq, bq, Wk, bk, Wv, bv, Wo, bo, **run_kwargs):
    B = np.asarray(x).shape[0]
    in_maps = _build_in_maps(x, Wq, bq, Wk, bk, Wv, bv, Wo, bo)
    res = bass_utils.run_bass_kernel_spmd(
        get_nc(), in_maps, core_ids=list(range(8)), **run_kwargs
    )
    if run_kwargs:
        kernel.last_result = res
    y = np.empty((B, S, D), np.float32)
    for c in range(8):
        b, half = c // 2, c % 2
        y[b, half * QC:(half + 1) * QC, :] = res.results[c]["y"]
    return y
